# revision 43
# baseline (speedup 1.0000x reference)
"""Trainium2 Bass kernel for nn_DBLossWithShift (fp8 DoubleRow rewrite).

Computes: mean((y_hat-y)^2) + 0.1 * min_{|d|<=5} mean((EMA(y_hat)[t+d]-EMA(y)[t])^2)
for y_hat, y of shape [128, 8192, 8] f32, EMA along t with alpha=0.2.

Strategy (data-parallel over batch, 8 cores x 16 batch elements):
  Host: per-core shard -> basis change S=y_hat+y, D=y_hat-y -> transpose to
  [t_in_block, block, bc] -> cast fp8 e4m3 -> interleave S/D blocks into ONE
  stream (chunk DMAs then unlock both streams in consumption order).  The
  EMA coefficient matrices ride as 4 extra block-slots at the head (saves a
  DMA: the 625ns HWDGE descriptor-generation cost per DMA is scarce).
  Device (per core), all matmuls fp8 DoubleRow (2 k-tiles per instr, 0.5
  cyc/row):
    - EMA per 128-t block j as ONE DR matmul: slotA=x_{j-1} x Aprev,
      slotB=x_j x Acur accumulated by the PE (2-block-banded truncation,
      q^128 ~ 4e-13).  E^T [bc, t] lands in PSUM f32.
    - PSUM -> SBUF fp8 evacuation of S_e, D_e (alternating ACT/DVE;
      Pool compute ops do not lower in this walrus build).
    - Block-accumulated Grams, DR-paired two blocks per matmul:
        G_SS[t',t] += sum_bc S_e S_e     G_DD += D_e D_e    G_X += D_e S_e
        G_db[bc,bc'] += sum_t d8 d8      (raw diff -> db loss via trace)
      All four accumulators share ONE PSUM bank: the bank is memset to zero
      once and every Gram matmul accumulates with start=False (a start=True
      would arm the whole 2KB zero-region and corrupt the neighbours).
  Host (f64): db = trace(G_db);
      Eh=(S_e+D_e)/2, Ee=(S_e-D_e)/2  =>
      corr(d) = 1/4 (A_SS(d) - A_DD(d) - X1(d) + X2(d)) from Gram diagonals,
      d2 = A_DD(0), normsum = d2 + 2 corr(0); shifted-MSE decomposition with
      exact head/tail edge corrections from the raw f32 inputs (same scheme
      as the bf16 baseline; block-boundary cross terms dropped, O(1e-4)
      relative on the shift loss).
"""

import sys

import numpy as np

for _p in ("/opt/trn_rl_repo",):
    if _p not in sys.path:
        sys.path.insert(0, _p)

import ml_dtypes

# ---------------------------------------------------------------- constants
B, T, C = 128, 8192, 8
NCORES = 8
BPC = B // NCORES          # 16 batch elements per core
P = 128                    # t-block size (partition dim)
NBLK = T // P              # 64 blocks
GRP = 4                    # blocks per PSUM group tile
NGRP = NBLK // GRP         # 16 groups
BC = BPC * C               # 128 channels per core (b*8 + c)
ALPHA = 0.2
LAM = 0.1
KSH = 5                    # max |shift|
NASLOT = 4                 # A-matrix slots prepended to the S stream
# Input DMA chunk sizes in slots (s includes the 4 A slots; sums 68 / 64).
# Graded: small first chunks prime the pipeline, the 625ns HWDGE generation
# cost per DMA caps how many chunks are worth it.
CHUNKS = (12, 8, 16, 24, 24, 24, 24)
GRAM_LAG = 2               # groups of lag before Gram matmuls are emitted

_F8 = ml_dtypes.float8_e4m3
_BF16 = ml_dtypes.bfloat16


def _ema_mats():
    """A matrices (f64): E^T = X_prev^T@Aprev + X_cur^T@Acur, [t_in, t_out]."""
    a = ALPHA
    q = 1.0 - a
    t_in = np.arange(P)[:, None]
    t_out = np.arange(P)[None, :]
    gap = t_out - t_in
    acur = np.where(gap >= 0, a * q ** np.clip(gap, 0, None), 0.0)
    a0 = acur.copy()
    a0[0, :] = q ** t_out[0]          # e_0 = x_0 boundary: coeff of x_0 is q^t
    gapp = t_out + P - t_in           # in [1, 255]
    aprev = a * q ** gapp
    return a0, acur, aprev


def _a_slots():
    """The 4 prepended S-stream slots: [Aprev, Acur, A0, 0], fp8."""
    a0_np, acur_np, aprev_np = _ema_mats()
    return np.stack(
        [aprev_np, acur_np, a0_np, np.zeros_like(a0_np)], axis=1).astype(_F8)


# ---------------------------------------------------------------- device IR
_MODULE_CACHE = {}


def _build_module():
    if "nc" in _MODULE_CACHE:
        return _MODULE_CACHE["nc"]
    from contextlib import ExitStack

    import concourse.tile as tile
    from concourse import bacc, mybir

    f32 = mybir.dt.float32
    bf16 = mybir.dt.bfloat16
    f8 = mybir.dt.float8e4
    DR = mybir.MatmulPerfMode.DoubleRow

    nc = bacc.Bacc("TRN2", target_bir_lowering=False, debug=False)
    x8_d = nc.dram_tensor("x8", [P, NASLOT + 2 * NBLK, BC], f8,
                          kind="ExternalInput")
    out_d = nc.dram_tensor("out", [P, 4, P], bf16, kind="ExternalOutput")

    x8_ap = x8_d.ap()

    with tile.TileContext(nc) as tc, ExitStack() as ctx:
        xpool = ctx.enter_context(tc.tile_pool(name="xin", bufs=1))
        epool = ctx.enter_context(tc.tile_pool(name="ecopy", bufs=1))
        pgrp = ctx.enter_context(
            tc.tile_pool(name="pgrp", bufs=2, space="PSUM"))
        pacc = ctx.enter_context(tc.tile_pool(name="pacc", bufs=1, space="PSUM"))

        xall = xpool.tile([P, NASLOT + 2 * NBLK, BC], f8, tag="xall")
        es = epool.tile([P, NBLK, BC], f8, tag="es")
        ed = epool.tile([P, NBLK, BC], f8, tag="ed")
        out_s = epool.tile([P, 4, P], bf16, tag="outs")

        rhs_norm = xall[:, 0:2, :]   # (Aprev, Acur)
        rhs_sj0 = xall[:, 1:3, :]    # (Acur, A0): pairs lhsT (Zero, s_0)
        rhs_dj0 = xall[:, 2:4, :]    # (A0, 0):    pairs lhsT (d_0, d_1)

        # All four Gram accumulators in one PSUM bank; zeroed once, then
        # accumulate-only matmuls (start=False).
        gall = pacc.tile([P, 512], f32, tag="gall")
        nc.vector.memset(gall[:], 0.0)
        g_ss = gall[:, 0:128]
        g_dd = gall[:, 128:256]
        g_x = gall[:, 256:384]
        g_db = gall[:, 384:512]

        # all input DMAs upfront (subtile deps gate the consumers)
        off = 0
        for w in CHUNKS:
            nc.sync.dma_start(xall[:, off:off + w, :], x8_ap[:, off:off + w, :])
            off += w
        assert off == NASLOT + 2 * NBLK

        # Pool DVE-style ops do not lower in this walrus build; evacs are
        # split between ACT (s stream) and DVE (d stream).
        evac_ops = {"act": nc.scalar.copy, "dve": nc.vector.tensor_copy}
        cycles = {"s": ("act", "dve"), "d": ("dve", "act")}
        evac_ks = {"s": 0, "d": 0}

        def evac(stream, dst, src, split=False):
            if split:
                h = GRP // 2
                evac_ops["act"](dst[:, 0:h, :], src[:, 0:h, :])
                evac_ops["dve"](dst[:, h:GRP, :], src[:, h:GRP, :])
                return
            cyc = cycles[stream]
            evac_ops[cyc[evac_ks[stream] % len(cyc)]](dst, src)
            evac_ks[stream] += 1

        def gram_mms(g):
            for m in (2 * g, 2 * g + 1):
                last = m == 2 * NGRP - 1
                sl = slice(2 * m, 2 * m + 2)
                nc.tensor.matmul(g_ss[:], es[:, sl, :], es[:, sl, :],
                                 start=False, stop=last, perf_mode=DR,
                                 skip_group_check=True)
                nc.tensor.matmul(g_dd[:], ed[:, sl, :], ed[:, sl, :],
                                 start=False, stop=last, perf_mode=DR,
                                 skip_group_check=True)
                nc.tensor.matmul(g_x[:], ed[:, sl, :], es[:, sl, :],
                                 start=False, stop=last, perf_mode=DR,
                                 skip_group_check=True)

        for g in range(NGRP):
            ps = pgrp.tile([P, GRP, P], f32, tag="ps", bufs=4)
            pd = pgrp.tile([P, GRP, P], f32, tag="pd", bufs=3)

            # EMA: one DoubleRow matmul per block per stream.
            # s_j at slot 4+2j, d_j at slot 5+2j (stride-2 pair APs).
            for st, pt in ((0, ps), (1, pd)):
                for s in range(GRP):
                    j = g * GRP + s
                    if j == 0 and st == 0:
                        lhs, rhs = xall[:, 3:5, :], rhs_sj0
                    elif j == 0:
                        lhs, rhs = xall[:, 5:8:2, :], rhs_dj0
                    else:
                        a = NASLOT + 2 * (j - 1) + st
                        lhs, rhs = xall[:, a:a + 3:2, :], rhs_norm
                    nc.tensor.matmul(pt[:, s:s + 1, :], lhs, rhs,
                                     start=True, stop=True, perf_mode=DR)

            # db self-Gram on raw d8 (PE filler while evac runs)
            for m in (2 * g, 2 * g + 1):
                sl = slice(NASLOT + 4 * m + 1, NASLOT + 4 * m + 4, 2)
                nc.tensor.matmul(g_db[:], xall[:, sl, :], xall[:, sl, :],
                                 start=False, stop=(m == 2 * NGRP - 1),
                                 perf_mode=DR, skip_group_check=True)

            # evacuate EMA PSUM -> SBUF fp8 (last group split for latency)
            last = g == NGRP - 1
            evac("s", es[:, g * GRP:(g + 1) * GRP, :], ps[:], split=last)
            evac("d", ed[:, g * GRP:(g + 1) * GRP, :], pd[:], split=last)

            # Grams lag so the PE never queues behind a fresh evac.
            if g >= GRAM_LAG:
                gram_mms(g - GRAM_LAG)
        for g in range(NGRP - GRAM_LAG, NGRP):
            gram_mms(g)

        # stage (f32 PSUM -> bf16 SBUF); a single ACT op — two engines
        # writing slices of one tile serialize on the tile dep anyway
        nc.scalar.copy(out_s[:], gall[:])
        nc.sync.dma_start(out_d.ap(), out_s[:])

    nc.compile()
    _MODULE_CACHE["nc"] = nc
    return nc


# ---------------------------------------------------------------- host side
def _shard_core(y_hat, y, core):
    """Per-core [16,8192,8] f32 -> x8 [P, 4+2*NBLK, BC] fp8.

    Slots: 0-3 = (Aprev, Acur, A0, 0); 4+2k = S block k; 5+2k = D block k.
    """
    yh = y_hat[core * BPC:(core + 1) * BPC].astype(np.float32)
    yy = y[core * BPC:(core + 1) * BPC].astype(np.float32)
    outs = []
    for arr in (yh + yy, yh - yy):
        x = arr.transpose(1, 0, 2).reshape(T, BC)           # [t, bc]
        x = x.reshape(NBLK, P, BC).transpose(1, 0, 2)       # [p, blk, bc]
        outs.append(x.astype(_F8))
    inter = np.stack(outs, axis=2).reshape(P, 2 * NBLK, BC)
    return np.ascontiguousarray(
        np.concatenate([_a_slots(), inter], axis=1))


def _emulate_core(x8_g):
    """Numpy emulation of the device kernel for one core (validation aid).

    x8_g: [P, 4+2*NBLK, BC] fp8 (A slots prepended, s/d interleaved).
    Returns the 4 Grams (f64).
    """
    a0 = x8_g[:, 2, :].astype(np.float32)
    ac = x8_g[:, 1, :].astype(np.float32)
    ap = x8_g[:, 0, :].astype(np.float32)

    def blocks(xg):
        return xg.astype(np.float32).transpose(1, 0, 2)     # [blk, t, bc]

    xs_b = blocks(x8_g[:, NASLOT::2, :])
    xd_b = blocks(x8_g[:, NASLOT + 1::2, :])

    def ema8(xb):
        e = np.zeros((NBLK, P, BC), np.float32)
        for j in range(NBLK):
            m = a0 if j == 0 else ac
            acc = m.T @ xb[j]
            if j > 0:
                acc = acc + ap.T @ xb[j - 1]
            e[j] = acc
        # device evacuates to fp8
        return e.astype(_F8).astype(np.float64)

    es_b, ed_b = ema8(xs_b), ema8(xd_b)

    # G[t',t] = sum_bc E[t',bc]E[t,bc], accumulated over blocks
    g_ss = np.zeros((P, P), np.float64)
    g_dd = np.zeros((P, P), np.float64)
    g_x = np.zeros((P, P), np.float64)
    for j in range(NBLK):
        g_ss += es_b[j] @ es_b[j].T
        g_dd += ed_b[j] @ ed_b[j].T
        g_x += ed_b[j] @ es_b[j].T
    xd64 = xd_b.astype(np.float64)
    g_db = np.einsum("jtb,jtc->bc", xd64, xd64)
    # device stages the Grams to bf16 before the output DMA
    return {k: v.astype(_BF16).astype(np.float64)
            for k, v in (("g_ss", g_ss), ("g_dd", g_dd),
                         ("g_x", g_x), ("g_db", g_db))}


def _host_edges(y_hat, y):
    """Exact (f64) head/tail EMA values of the full batch.

    Returns (head_h, head_e, tail_h, tail_e): each [KSH, B, C] f64 where
    head_*[k] = EMA[t=k], tail_*[k] = EMA[t=T-KSH+k].
    """
    a, q = ALPHA, 1.0 - ALPHA
    heads, tails = [], []
    for arr in (y_hat, y):
        x = arr.astype(np.float64)
        e = x[:, 0, :]
        hh = [e]
        for t in range(1, KSH):
            e = a * x[:, t, :] + q * e
            hh.append(e)
        heads.append(np.stack(hh))
        wtail = 700
        e = np.zeros_like(x[:, 0, :])
        tt = {}
        for t in range(T - wtail, T):
            e = a * x[:, t, :] + q * e
            if t >= T - KSH:
                tt[t] = e
        tails.append(np.stack([tt[T - KSH + k] for k in range(KSH)]))
    return heads[0], heads[1], tails[0], tails[1]


def _host_reduce(gsum, y_hat, y):
    """Assemble the final scalar loss (f64) from summed Grams."""
    head_h, head_e, tail_h, tail_e = _host_edges(y_hat, y)
    hh2 = (head_h ** 2).sum(axis=(1, 2))   # [KSH] per-t sums
    he2 = (head_e ** 2).sum(axis=(1, 2))
    th2 = (tail_h ** 2).sum(axis=(1, 2))
    te2 = (tail_e ** 2).sum(axis=(1, 2))

    def diag(gm, d):
        # sum_t M[t+d, t]
        return np.diagonal(gm, offset=-d).sum()

    corr = {}
    for d in range(-KSH, KSH + 1):
        a_ss = diag(gsum["g_ss"], d)
        a_dd = diag(gsum["g_dd"], d)
        x2 = diag(gsum["g_x"], d)      # sum D_e[t+d] S_e[t]
        x1 = diag(gsum["g_x"], -d)     # sum S_e[t+d] D_e[t]
        corr[d] = 0.25 * (a_ss - a_dd - x1 + x2)

    d2_num = diag(gsum["g_dd"], 0)
    normsum = d2_num + 2.0 * corr[0]

    errs = []
    for d in range(-KSH, KSH + 1):
        nd = B * C * (T - abs(d))
        if d >= 0:
            head_cut = hh2[:d].sum() if d > 0 else 0.0
            tail_cut = te2[KSH - d:].sum() if d > 0 else 0.0
        else:
            s = -d
            head_cut = he2[:s].sum()
            tail_cut = th2[KSH - s:].sum()
        num = normsum - head_cut - tail_cut - 2.0 * corr[d]
        errs.append(num / nd)

    db_loss = np.trace(gsum["g_db"]) / (B * T * C)
    return db_loss + LAM * min(errs)


def _run_device(y_hat, y, trace=False):
    """Build shards, run the SPMD kernel, return per-core result dicts."""
    from concourse.bass_utils import run_bass_kernel_spmd

    nc = _build_module()
    in_maps = []
    for core in range(NCORES):
        in_maps.append({"x8": _shard_core(y_hat, y, core)})
    res = run_bass_kernel_spmd(
        nc, in_maps, core_ids=list(range(NCORES)), trace=trace,
    )
    return res


def _sum_grams(results):
    keys = ("g_ss", "g_dd", "g_x", "g_db")
    gsum = {k: np.zeros((P, P), np.float64) for k in keys}
    for r in results:
        out = r["out"]
        for i, k in enumerate(keys):
            gsum[k] += out[:, i, :].astype(np.float64)
    return gsum


def kernel(y_hat, y):
    res = _run_device(y_hat, y, trace=False)
    gsum = _sum_grams(res.results)
    return np.float32(_host_reduce(gsum, y_hat, y))
